# revision 16
# baseline (speedup 1.0000x reference)
"""CombinedMarginLoss (ArcFace, m1=1, m2=0.5, m3=0, easy_margin) on 8 trn2 cores.

Math: loss = mean_b [ logsumexp_c(margin_logits[b,c]) - S*theta_b ] where
margin_logits[b,c] = S*logits[b,c] except the label column which is S*theta_b.

Because logits are cosine similarities in [-1, 1], S*x - S lies in [-128, 0],
so exp(S*x - S) never overflows in fp32 and the per-row sum-exp needs no max
pass: a single DMA-bound sweep per core suffices.  The class dimension is
sharded across the 8 cores (partial-FC style); each core returns its partial
per-row sum of exp(S*x - S).  The O(B) label gather, margin transform, and
log/mean epilogue are done on the host as part of unsharding.

Optimizations:
- Inputs are shipped to the device as fp16 (logits are in [-1,1]; the fp16
  rounding jitter of +-1.6% per exp term averages out over the ~1e3
  effective softmax terms per row, final loss error ~1e-5 relative).
- Host packs each core's shard into a flat buffer of [128, W] chunk blobs so
  every DMA reads one fully contiguous region at max HBM bandwidth.
- exp is computed 70% on ScalarE (hardware Exp with fused per-partition
  accum_out) and 30% on VectorE via the Schraudolph bit-trick
  (int32(A*x+B) reinterpreted as fp32 ~= exp(S*x-S)), whose +1.07% bias is
  removed by a calibrated host-side gamma. Both engines then hide entirely
  under the DMA stream.
- Values below the clamp (-0.25, i.e. exp < 2e-35) cannot affect the sum;
  the host clamps so the bit-trick's int never goes negative.
"""

import numpy as np

_S = 64.0
_M2 = 0.5
_EPS = 1e-7
_NCORES = 8
_P = 128  # SBUF partitions

_CLAMP = -0.25  # exp(64*-0.25 - 64) = 1.8e-35: far below fp32 sum resolution

_LOG2E = 1.4426950408889634
# bf16 variant of the bit trick: bf16 has fp32's 8-bit exponent, so
# int16(A*x + B) bitcast to bf16 ~= exp(S*x - S); int16 output lets the
# tensor_scalar run in the DVE 4x mode and bf16 tensor_tensor folds run 2x.
_SCH_A = _S * _LOG2E * 2.0**7
_SCH_C = 0.0434609
_SCH_B = 2.0**7 * (127.0 - _S * _LOG2E - _SCH_C)
# E[bit-trick exp / true exp] under exp-weighted uniform inputs; calibrated
# against float64 on-device (see calib.py); host divides it back out.
_GAMMA = 0.99029446  # HW-calibrated (CoreSim value differs: 0.99284518)

# per 128-row block: (width, engine) chunk list; class dim = 12500 per core.
# 52% ScalarE / 48% VectorE; small trailing ACT chunk trims the kernel tail.
_CHUNKS_12500 = [(3000, "D"), (2750, "A"), (3000, "D"), (2750, "A"), (1000, "A")]
# DVE implementation: "ttr" (tensor_tensor_reduce fold) | "fold" (tensor_tensor
# adds + reduce) | "i32red" (int32 bit-trick + fp32 reduce, the v4 path).
# NOTE: "ttr" with bf16 operands passes CoreSim but faults TRN2 hardware
# (NRT_EXEC_UNIT_UNRECOVERABLE) — do not use.
_DVE_IMPL = "fold"

_nc_cache = {}


def _chunk_plan(Cs):
    if Cs % 12500 == 0:
        return _CHUNKS_12500 * (Cs // 12500)
    # fallback: uniform ~6250-wide ACT-only chunks
    n = max(1, -(-Cs // 6250))
    while Cs % n:
        n += 1
    return [(Cs // n, "A")] * n


def _build_nc(B, Cs):
    """Bass/Tile program for one core: xflat[B*Cs] fp16 (blob layout) ->
    sums[128, 2*nblk]; col blk = ScalarE partial, col nblk+blk = VectorE
    (bit-trick, pre-gamma) partial of sum_c exp(S*x[blk*128+p, c] - S)."""
    import concourse.bacc as bacc
    import concourse.mybir as mybir
    from concourse.tile import TileContext

    nblk = B // _P
    plan = _chunk_plan(Cs)
    nch = len(plan)
    n_act = sum(1 for _, e in plan if e == "A")
    wmax = max(w for w, _ in plan)
    wmax_d = max([w for w, e in plan if e == "D"] or [1])

    nc = bacc.Bacc("TRN2", target_bir_lowering=False)
    x = nc.dram_tensor("x", [B * Cs], mybir.dt.float16, kind="ExternalInput")
    out = nc.dram_tensor(
        "sums", [_P, 2 * nblk], mybir.dt.float32, kind="ExternalOutput"
    )

    with TileContext(nc) as tc:
        with (
            tc.tile_pool(name="inp", bufs=10) as inp,
            tc.tile_pool(name="scr", bufs=2) as scr,
            tc.tile_pool(name="acc", bufs=1) as accp,
        ):
            bias = accp.tile([_P, 1], mybir.dt.float32)
            nc.gpsimd.memset(bias[:], -_S)
            acc = accp.tile([_P, nblk * n_act], mybir.dt.float32)
            res = accp.tile([_P, 2 * nblk], mybir.dt.float32)
            off = 0
            for blk in range(nblk):
                ia = 0
                for W, eng in plan:
                    t = inp.tile([_P, wmax], mybir.dt.float16, tag="inp")
                    nc.sync.dma_start(
                        out=t[:, :W],
                        in_=x[off : off + _P * W].rearrange("(p w) -> p w", p=_P),
                    )
                    if eng == "A":
                        s = scr.tile([_P, wmax], mybir.dt.float16, tag="scr")
                        # s = exp(S*t - S); acc col = per-partition row-sum
                        nc.scalar.activation(
                            out=s[:, :W],
                            in_=t[:, :W],
                            func=mybir.ActivationFunctionType.Exp,
                            scale=_S,
                            bias=bias[:],
                            accum_out=acc[:, blk * n_act + ia : blk * n_act + ia + 1],
                        )
                        ia += 1
                    elif _DVE_IMPL == "i32red":
                        i32 = scr.tile([_P, wmax_d], mybir.dt.int32, tag="i32")
                        # int32(A32*x + B32) bit pattern ~= fp32 exp(S*x - S)
                        nc.vector.tensor_scalar(
                            out=i32[:, :W],
                            in0=t[:, :W],
                            scalar1=_SCH_A * 2.0**16,
                            scalar2=_SCH_B * 2.0**16,
                            op0=mybir.AluOpType.mult,
                            op1=mybir.AluOpType.add,
                        )
                        nc.vector.reduce_sum(
                            out=res[:, nblk + blk : nblk + blk + 1],
                            in_=i32[:, :W].bitcast(mybir.dt.float32),
                            axis=mybir.AxisListType.X,
                        )
                    else:
                        assert W % 4 == 0
                        i16 = scr.tile([_P, wmax_d], mybir.dt.int16, tag="i16")
                        # int16(A*x + B) bit pattern ~= bf16 exp(S*x - S)
                        nc.vector.tensor_scalar(
                            out=i16[:, :W],
                            in0=t[:, :W],
                            scalar1=_SCH_A,
                            scalar2=_SCH_B,
                            op0=mybir.AluOpType.mult,
                            op1=mybir.AluOpType.add,
                        )
                        bf = i16[:, :W].bitcast(mybir.dt.bfloat16)
                        h = W // 2
                        q = W // 4
                        if _DVE_IMPL == "ttr":
                            f1 = scr.tile(
                                [_P, wmax_d // 2], mybir.dt.bfloat16, tag="f1"
                            )
                            # f1 = bf_lo + bf_hi; accum = row-sum(f1) (one DVE op)
                            nc.vector.tensor_tensor_reduce(
                                out=f1[:, :h],
                                in0=bf[:, :h],
                                in1=bf[:, h:],
                                scale=1.0,
                                scalar=0.0,
                                op0=mybir.AluOpType.add,
                                op1=mybir.AluOpType.add,
                                accum_out=res[:, nblk + blk : nblk + blk + 1],
                            )
                        else:
                            f1 = scr.tile(
                                [_P, wmax_d // 2], mybir.dt.bfloat16, tag="f1"
                            )
                            nc.vector.tensor_tensor(
                                out=f1[:, :h],
                                in0=bf[:, :h],
                                in1=bf[:, h:],
                                op=mybir.AluOpType.add,
                            )
                            f2 = scr.tile(
                                [_P, wmax_d // 4], mybir.dt.bfloat16, tag="f2"
                            )
                            nc.vector.tensor_tensor(
                                out=f2[:, :q],
                                in0=f1[:, :q],
                                in1=f1[:, q : 2 * q],
                                op=mybir.AluOpType.add,
                            )
                            nc.vector.reduce_sum(
                                out=res[:, nblk + blk : nblk + blk + 1],
                                in_=f2[:, :q],
                                axis=mybir.AxisListType.X,
                            )
                    off += _P * W
            for blk in range(nblk):
                nc.vector.reduce_sum(
                    out=res[:, blk : blk + 1],
                    in_=acc[:, blk * n_act : (blk + 1) * n_act],
                    axis=mybir.AxisListType.X,
                )
            nc.sync.dma_start(out=out[:], in_=res[:])

    nc.compile()
    return nc


def _get_nc(B, Cs):
    key = (B, Cs)
    if key not in _nc_cache:
        _nc_cache[key] = _build_nc(B, Cs)
    return _nc_cache[key]


def _pack_shard(shard_f16, plan):
    """[B, Cs] fp16 -> flat blob layout matching _build_nc's DMA order."""
    B, Cs = shard_f16.shape
    parts = []
    for blk in range(B // _P):
        off = 0
        rows = shard_f16[blk * _P : (blk + 1) * _P]
        for W, _ in plan:
            parts.append(rows[:, off : off + W].ravel())
            off += W
    return np.concatenate(parts)


def _device_row_sums(logits, trace=False):
    """Shard the class dim over 8 cores, run the bass kernel, return
    (row_sums[B] float64 = sum_c exp(S*logits - S), BassKernelResults)."""
    from concourse.bass_utils import run_bass_kernel_spmd

    B, C = logits.shape
    Bp = -(-B // _P) * _P  # pad rows to a multiple of 128
    Cp = -(-C // _NCORES) * _NCORES  # pad cols to a multiple of 8
    x16 = np.maximum(logits, _CLAMP).astype(np.float16)
    if Bp != B or Cp != C:
        padded = np.full((Bp, Cp), _CLAMP, dtype=np.float16)
        padded[:B, :C] = x16
        x16 = padded
    Cs = Cp // _NCORES
    plan = _chunk_plan(Cs)
    nblk = Bp // _P
    nc = _get_nc(Bp, Cs)
    in_maps = [
        {"x": _pack_shard(x16[:, i * Cs : (i + 1) * Cs], plan)} for i in range(_NCORES)
    ]
    r = run_bass_kernel_spmd(nc, in_maps, core_ids=list(range(_NCORES)), trace=trace)
    total = np.zeros(Bp, np.float64)
    for res in r.results:
        arr = res["sums"].astype(np.float64)  # [128, 2*nblk]
        act = arr[:, :nblk].T.reshape(Bp)
        dve = arr[:, nblk:].T.reshape(Bp)
        total += act + _GAMMA * dve
    # The clamp floor contributes ~1.8e-35 per clamped element on the ACT
    # side and ~0 on the DVE side; both are below fp32 resolution of the
    # per-row sums (>= exp(0) for a max-logit near 1), so no correction.
    return total[:B], r


def kernel(logits, labels):
    logits = np.ascontiguousarray(np.asarray(logits, dtype=np.float32))
    labels_i = np.asarray(labels).astype(np.int64)
    B, C = logits.shape

    total, _ = _device_row_sums(logits)

    rows = np.arange(B)
    t = logits[rows, labels_i].astype(np.float64)
    # subtract what the device actually added for the label column (its fp16
    # value); the margin math itself uses the exact fp32 target.
    t16 = t.astype(np.float16).astype(np.float64)
    thresh = float(np.cos(np.pi - _M2))
    ang = np.arccos(np.clip(t, -1.0 + _EPS, 1.0 - _EPS))
    cos_m = np.cos(ang + _M2)
    theta = np.where(t > thresh, cos_m, -2.0 - cos_m)

    # replace the label column's exp term, all under the constant shift S
    corrected = total - np.exp(_S * t16 - _S) + np.exp(_S * theta - _S)
    loss_rows = _S + np.log(corrected) - _S * theta
    return np.array(loss_rows.mean(), dtype=np.float32)


# revision 18
# speedup vs baseline: 1.0614x; 1.0614x over previous
"""CombinedMarginLoss (ArcFace, m1=1, m2=0.5, m3=0, easy_margin) on 8 trn2 cores.

Math: loss = mean_b [ logsumexp_c(margin_logits[b,c]) - S*theta_b ] where
margin_logits[b,c] = S*logits[b,c] except the label column which is S*theta_b.

Because logits are cosine similarities in [-1, 1], S*x - S lies in [-128, 0],
so exp(S*x - S) never overflows in fp32 and the per-row sum-exp needs no max
pass: a single DMA-bound sweep per core suffices.  The class dimension is
sharded across the 8 cores (partial-FC style); each core returns its partial
per-row sum of exp(S*x - S).  The O(B) label gather, margin transform, and
log/mean epilogue are done on the host as part of unsharding.

Optimizations:
- Inputs are shipped to the device as fp16 (logits are in [-1,1]; the fp16
  rounding jitter of +-1.6% per exp term averages out over the ~1e3
  effective softmax terms per row, final loss error ~1e-5 relative).
- Host packs each core's shard into a flat buffer of [128, W] chunk blobs so
  every DMA reads one fully contiguous region at max HBM bandwidth.
- exp is computed 70% on ScalarE (hardware Exp with fused per-partition
  accum_out) and 30% on VectorE via the Schraudolph bit-trick
  (int32(A*x+B) reinterpreted as fp32 ~= exp(S*x-S)), whose +1.07% bias is
  removed by a calibrated host-side gamma. Both engines then hide entirely
  under the DMA stream.
- Values below the clamp (-0.25, i.e. exp < 2e-35) cannot affect the sum;
  the host clamps so the bit-trick's int never goes negative.
"""

import numpy as np

_S = 64.0
_M2 = 0.5
_EPS = 1e-7
_NCORES = 8
_P = 128  # SBUF partitions

_CLAMP = -0.25  # exp(64*-0.25 - 64) = 1.8e-35: far below fp32 sum resolution

_LOG2E = 1.4426950408889634
# bf16 variant of the bit trick: bf16 has fp32's 8-bit exponent, so
# int16(A*x + B) bitcast to bf16 ~= exp(S*x - S); int16 output lets the
# tensor_scalar run in the DVE 4x mode and bf16 tensor_tensor folds run 2x.
_SCH_A = _S * _LOG2E * 2.0**7
_SCH_C = 0.0434609
_SCH_B = 2.0**7 * (127.0 - _S * _LOG2E - _SCH_C)
# E[bit-trick exp / true exp] under exp-weighted uniform inputs; calibrated
# against float64 on-device (see calib.py); host divides it back out.
_GAMMA = 0.99029446  # HW-calibrated (CoreSim value differs: 0.99284518)

# per 128-row block: (width, engine) chunk list; class dim = 12500 per core.
# 52% ScalarE / 48% VectorE; small trailing ACT chunk trims the kernel tail.
_CHUNKS_12500 = [(3000, "D"), (2750, "A"), (3000, "D"), (2750, "A"), (1000, "A")]
# DVE implementation: "ttr" (tensor_tensor_reduce fold) | "fold" (tensor_tensor
# adds + reduce) | "i32red" (int32 bit-trick + fp32 reduce, the v4 path).
# NOTE: "ttr" with bf16 operands passes CoreSim but faults TRN2 hardware
# (NRT_EXEC_UNIT_UNRECOVERABLE) — do not use.
_DVE_IMPL = "fold"

_nc_cache = {}


def _chunk_plan(Cs):
    if Cs % 12500 == 0:
        return _CHUNKS_12500 * (Cs // 12500)
    # fallback: uniform ~6250-wide ACT-only chunks
    n = max(1, -(-Cs // 6250))
    while Cs % n:
        n += 1
    return [(Cs // n, "A")] * n


def _build_nc(B, Cs):
    """Bass/Tile program for one core: xflat[B*Cs] fp16 (blob layout) ->
    sums[128, 2*nblk]; col blk = ScalarE partial, col nblk+blk = VectorE
    (bit-trick, pre-gamma) partial of sum_c exp(S*x[blk*128+p, c] - S)."""
    import concourse.bacc as bacc
    import concourse.mybir as mybir
    from concourse.tile import TileContext

    nblk = B // _P
    plan = _chunk_plan(Cs)
    nch = len(plan)
    n_act = sum(1 for _, e in plan if e == "A")
    n_dve = sum(1 for _, e in plan if e == "D")
    wmax = max(w for w, _ in plan)
    wmax_d = max([w for w, e in plan if e == "D"] or [1])

    nc = bacc.Bacc("TRN2", target_bir_lowering=False)
    x = nc.dram_tensor("x", [B * Cs], mybir.dt.float16, kind="ExternalInput")
    out = nc.dram_tensor(
        "sums", [_P, nblk * (1 + n_dve)], mybir.dt.float32, kind="ExternalOutput"
    )

    with TileContext(nc) as tc:
        with (
            tc.tile_pool(name="inp", bufs=10) as inp,
            tc.tile_pool(name="scr", bufs=3) as scr,
            tc.tile_pool(name="acc", bufs=1) as accp,
        ):
            bias = accp.tile([_P, 1], mybir.dt.float32)
            nc.gpsimd.memset(bias[:], -_S)
            acc = accp.tile([_P, nblk * n_act], mybir.dt.float32)
            res = accp.tile([_P, nblk * (1 + n_dve)], mybir.dt.float32)
            off = 0
            for blk in range(nblk):
                ia = 0
                idv = 0
                for W, eng in plan:
                    t = inp.tile([_P, wmax], mybir.dt.float16, tag="inp")
                    nc.sync.dma_start(
                        out=t[:, :W],
                        in_=x[off : off + _P * W].rearrange("(p w) -> p w", p=_P),
                    )
                    if eng == "A":
                        s = scr.tile([_P, wmax], mybir.dt.float16, tag="scr")
                        # s = exp(S*t - S); acc col = per-partition row-sum
                        nc.scalar.activation(
                            out=s[:, :W],
                            in_=t[:, :W],
                            func=mybir.ActivationFunctionType.Exp,
                            scale=_S,
                            bias=bias[:],
                            accum_out=acc[:, blk * n_act + ia : blk * n_act + ia + 1],
                        )
                        ia += 1
                    elif _DVE_IMPL == "i32red":
                        i32 = scr.tile([_P, wmax_d], mybir.dt.int32, tag="i32")
                        # int32(A32*x + B32) bit pattern ~= fp32 exp(S*x - S)
                        nc.vector.tensor_scalar(
                            out=i32[:, :W],
                            in0=t[:, :W],
                            scalar1=_SCH_A * 2.0**16,
                            scalar2=_SCH_B * 2.0**16,
                            op0=mybir.AluOpType.mult,
                            op1=mybir.AluOpType.add,
                        )
                        sl = nblk + blk * n_dve + idv
                        idv += 1
                        nc.vector.reduce_sum(
                            out=res[:, sl : sl + 1],
                            in_=i32[:, :W].bitcast(mybir.dt.float32),
                            axis=mybir.AxisListType.X,
                        )
                    else:
                        assert W % 4 == 0
                        i16 = scr.tile([_P, wmax_d], mybir.dt.int16, tag="i16")
                        # int16(A*x + B) bit pattern ~= bf16 exp(S*x - S)
                        nc.vector.tensor_scalar(
                            out=i16[:, :W],
                            in0=t[:, :W],
                            scalar1=_SCH_A,
                            scalar2=_SCH_B,
                            op0=mybir.AluOpType.mult,
                            op1=mybir.AluOpType.add,
                        )
                        bf = i16[:, :W].bitcast(mybir.dt.bfloat16)
                        h = W // 2
                        q = W // 4
                        sl = nblk + blk * n_dve + idv
                        idv += 1
                        if _DVE_IMPL == "ttr":
                            f1 = scr.tile(
                                [_P, wmax_d // 2], mybir.dt.bfloat16, tag="f1"
                            )
                            # f1 = bf_lo + bf_hi; accum = row-sum(f1) (one DVE op)
                            nc.vector.tensor_tensor_reduce(
                                out=f1[:, :h],
                                in0=bf[:, :h],
                                in1=bf[:, h:],
                                scale=1.0,
                                scalar=0.0,
                                op0=mybir.AluOpType.add,
                                op1=mybir.AluOpType.add,
                                accum_out=res[:, sl : sl + 1],
                            )
                        else:
                            f1 = scr.tile(
                                [_P, wmax_d // 2], mybir.dt.bfloat16, tag="f1"
                            )
                            nc.vector.tensor_tensor(
                                out=f1[:, :h],
                                in0=bf[:, :h],
                                in1=bf[:, h:],
                                op=mybir.AluOpType.add,
                            )
                            f2 = scr.tile(
                                [_P, wmax_d // 4], mybir.dt.bfloat16, tag="f2"
                            )
                            nc.vector.tensor_tensor(
                                out=f2[:, :q],
                                in0=f1[:, :q],
                                in1=f1[:, q : 2 * q],
                                op=mybir.AluOpType.add,
                            )
                            nc.vector.reduce_sum(
                                out=res[:, sl : sl + 1],
                                in_=f2[:, :q],
                                axis=mybir.AxisListType.X,
                            )
                    off += _P * W
            for blk in range(nblk):
                nc.vector.reduce_sum(
                    out=res[:, blk : blk + 1],
                    in_=acc[:, blk * n_act : (blk + 1) * n_act],
                    axis=mybir.AxisListType.X,
                )
            nc.sync.dma_start(out=out[:], in_=res[:])

    nc.compile()
    return nc


def _get_nc(B, Cs):
    key = (B, Cs)
    if key not in _nc_cache:
        _nc_cache[key] = _build_nc(B, Cs)
    return _nc_cache[key]


def _pack_shard(shard_f16, plan):
    """[B, Cs] fp16 -> flat blob layout matching _build_nc's DMA order."""
    B, Cs = shard_f16.shape
    parts = []
    for blk in range(B // _P):
        off = 0
        rows = shard_f16[blk * _P : (blk + 1) * _P]
        for W, _ in plan:
            parts.append(rows[:, off : off + W].ravel())
            off += W
    return np.concatenate(parts)


def _device_row_sums(logits, trace=False):
    """Shard the class dim over 8 cores, run the bass kernel, return
    (row_sums[B] float64 = sum_c exp(S*logits - S), BassKernelResults)."""
    from concourse.bass_utils import run_bass_kernel_spmd

    B, C = logits.shape
    Bp = -(-B // _P) * _P  # pad rows to a multiple of 128
    Cp = -(-C // _NCORES) * _NCORES  # pad cols to a multiple of 8
    x16 = np.maximum(logits, _CLAMP).astype(np.float16)
    if Bp != B or Cp != C:
        padded = np.full((Bp, Cp), _CLAMP, dtype=np.float16)
        padded[:B, :C] = x16
        x16 = padded
    Cs = Cp // _NCORES
    plan = _chunk_plan(Cs)
    n_dve = sum(1 for _, e in plan if e == "D")
    nblk = Bp // _P
    nc = _get_nc(Bp, Cs)
    in_maps = [
        {"x": _pack_shard(x16[:, i * Cs : (i + 1) * Cs], plan)} for i in range(_NCORES)
    ]
    r = run_bass_kernel_spmd(nc, in_maps, core_ids=list(range(_NCORES)), trace=trace)
    total = np.zeros(Bp, np.float64)
    for res in r.results:
        arr = res["sums"].astype(np.float64)  # [128, nblk*(1+n_dve)]
        act = arr[:, :nblk].T.reshape(Bp)
        dve = arr[:, nblk:].reshape(_P, nblk, n_dve).sum(axis=2).T.reshape(Bp)
        total += act + _GAMMA * dve
    # The clamp floor contributes ~1.8e-35 per clamped element on the ACT
    # side and ~0 on the DVE side; both are below fp32 resolution of the
    # per-row sums (>= exp(0) for a max-logit near 1), so no correction.
    return total[:B], r


def kernel(logits, labels):
    logits = np.ascontiguousarray(np.asarray(logits, dtype=np.float32))
    labels_i = np.asarray(labels).astype(np.int64)
    B, C = logits.shape

    total, _ = _device_row_sums(logits)

    rows = np.arange(B)
    t = logits[rows, labels_i].astype(np.float64)
    # subtract what the device actually added for the label column (its fp16
    # value); the margin math itself uses the exact fp32 target.
    t16 = t.astype(np.float16).astype(np.float64)
    thresh = float(np.cos(np.pi - _M2))
    ang = np.arccos(np.clip(t, -1.0 + _EPS, 1.0 - _EPS))
    cos_m = np.cos(ang + _M2)
    theta = np.where(t > thresh, cos_m, -2.0 - cos_m)

    # replace the label column's exp term, all under the constant shift S
    corrected = total - np.exp(_S * t16 - _S) + np.exp(_S * theta - _S)
    loss_rows = _S + np.log(corrected) - _S * theta
    return np.array(loss_rows.mean(), dtype=np.float32)


# revision 19
# speedup vs baseline: 1.0755x; 1.0132x over previous
"""CombinedMarginLoss (ArcFace, m1=1, m2=0.5, m3=0, easy_margin) on 8 trn2 cores.

Math: loss = mean_b [ logsumexp_c(margin_logits[b,c]) - S*theta_b ] where
margin_logits[b,c] = S*logits[b,c] except the label column which is S*theta_b.

Because logits are cosine similarities in [-1, 1], S*x - S lies in [-128, 0],
so exp(S*x - S) never overflows in fp32 and the per-row sum-exp needs no max
pass: a single DMA-bound sweep per core suffices.  The class dimension is
sharded across the 8 cores (partial-FC style); each core returns its partial
per-row sum of exp(S*x - S).  The O(B) label gather, margin transform, and
log/mean epilogue are done on the host as part of unsharding.

Optimizations:
- Inputs are shipped to the device as fp16 (logits are in [-1,1]; the fp16
  rounding jitter of +-1.6% per exp term averages out over the ~1e3
  effective softmax terms per row, final loss error ~1e-5 relative).
- Host packs each core's shard into a flat buffer of [128, W] chunk blobs so
  every DMA reads one fully contiguous region at max HBM bandwidth.
- exp is computed 70% on ScalarE (hardware Exp with fused per-partition
  accum_out) and 30% on VectorE via the Schraudolph bit-trick
  (int32(A*x+B) reinterpreted as fp32 ~= exp(S*x-S)), whose +1.07% bias is
  removed by a calibrated host-side gamma. Both engines then hide entirely
  under the DMA stream.
- Values below the clamp (-0.25, i.e. exp < 2e-35) cannot affect the sum;
  the host clamps so the bit-trick's int never goes negative.
"""

import numpy as np

_S = 64.0
_M2 = 0.5
_EPS = 1e-7
_NCORES = 8
_P = 128  # SBUF partitions

_CLAMP = -0.25  # exp(64*-0.25 - 64) = 1.8e-35: far below fp32 sum resolution

_LOG2E = 1.4426950408889634
# bf16 variant of the bit trick: bf16 has fp32's 8-bit exponent, so
# int16(A*x + B) bitcast to bf16 ~= exp(S*x - S); int16 output lets the
# tensor_scalar run in the DVE 4x mode and bf16 tensor_tensor folds run 2x.
_SCH_A = _S * _LOG2E * 2.0**7
_SCH_C = 0.0434609
_SCH_B = 2.0**7 * (127.0 - _S * _LOG2E - _SCH_C)
# E[bit-trick exp / true exp] under exp-weighted uniform inputs; calibrated
# against float64 on-device (see calib.py); host divides it back out.
_GAMMA = 0.99029446  # HW-calibrated (CoreSim value differs: 0.99284518)

# per 128-row block: (width, engine) chunk list; class dim = 12500 per core.
# 52% ScalarE / 48% VectorE; small trailing ACT chunk trims the kernel tail.
_CHUNKS_12500 = [(3000, "D"), (2750, "A"), (3000, "D"), (2750, "A"), (1000, "A")]


def _global_plan(nblk, Cs):
    """DMA-ordered list of (blk, W, eng).  Big chunks stream first; two small
    ACT chunks land last so both engines drain as the stream ends."""
    if Cs == 12500 and nblk == 4:
        acfg = {0: [6500], 1: [6500], 2: [5500, 1000], 3: [5500, 1000]}
        order = []
        for blk in range(nblk):
            order.append((blk, acfg[blk][0], "A"))
            order.append((blk, 3000, "D"))
            order.append((blk, 3000, "D"))
        order.append((2, 1000, "A"))
        order.append((3, 1000, "A"))
        return order
    return [(blk, W, e) for blk in range(nblk) for (W, e) in _chunk_plan(Cs)]
# DVE implementation: "ttr" (tensor_tensor_reduce fold) | "fold" (tensor_tensor
# adds + reduce) | "i32red" (int32 bit-trick + fp32 reduce, the v4 path).
# NOTE: "ttr" with bf16 operands passes CoreSim but faults TRN2 hardware
# (NRT_EXEC_UNIT_UNRECOVERABLE) — do not use.
_DVE_IMPL = "fold"

_nc_cache = {}


def _chunk_plan(Cs):
    if Cs % 12500 == 0:
        return _CHUNKS_12500 * (Cs // 12500)
    # fallback: uniform ~6250-wide ACT-only chunks
    n = max(1, -(-Cs // 6250))
    while Cs % n:
        n += 1
    return [(Cs // n, "A")] * n


def _build_nc(B, Cs):
    """Bass/Tile program for one core: xflat[B*Cs] fp16 (blob layout) ->
    sums[128, nblk*(1+n_dve)]; col blk = ScalarE partial, col nblk+blk*n_dve+i
    = VectorE (bit-trick, pre-gamma) partials of sum_c exp(S*x[...] - S)."""
    import concourse.bacc as bacc
    import concourse.mybir as mybir
    from concourse.tile import TileContext

    nblk = B // _P
    plan = _global_plan(nblk, Cs)
    n_act_by_blk = [sum(1 for b, _, e in plan if b == k and e == "A") for k in range(nblk)]
    n_dve_by_blk = [sum(1 for b, _, e in plan if b == k and e == "D") for k in range(nblk)]
    n_dve = n_dve_by_blk[0]
    assert all(v == n_dve for v in n_dve_by_blk)
    acc_base = [sum(n_act_by_blk[:k]) for k in range(nblk)]
    n_acc = sum(n_act_by_blk)
    wmax = max(w for _, w, _ in plan)
    wmax_d = max([w for _, w, e in plan if e == "D"] or [1])

    nc = bacc.Bacc("TRN2", target_bir_lowering=False)
    x = nc.dram_tensor("x", [B * Cs], mybir.dt.float16, kind="ExternalInput")
    out = nc.dram_tensor(
        "sums", [_P, nblk * (1 + n_dve)], mybir.dt.float32, kind="ExternalOutput"
    )

    with TileContext(nc) as tc:
        with (
            tc.tile_pool(name="inp", bufs=10) as inp,
            tc.tile_pool(name="scr", bufs=3) as scr,
            tc.tile_pool(name="acc", bufs=1) as accp,
        ):
            bias = accp.tile([_P, 1], mybir.dt.float32)
            nc.gpsimd.memset(bias[:], -_S)
            acc = accp.tile([_P, max(n_acc, 1)], mybir.dt.float32)
            res = accp.tile([_P, nblk * (1 + n_dve)], mybir.dt.float32)
            ia = [0] * nblk
            idv = [0] * nblk
            off = 0
            for blk, W, eng in plan:
                t = inp.tile([_P, wmax], mybir.dt.float16, tag="inp")
                nc.sync.dma_start(
                    out=t[:, :W],
                    in_=x[off : off + _P * W].rearrange("(p w) -> p w", p=_P),
                )
                if eng == "A":
                    s = scr.tile([_P, wmax], mybir.dt.float16, tag="scr")
                    if n_act_by_blk[blk] == 1:
                        dst = res[:, blk : blk + 1]
                    else:
                        j = acc_base[blk] + ia[blk]
                        dst = acc[:, j : j + 1]
                    ia[blk] += 1
                    # s = exp(S*t - S); dst = per-partition row-sum of s
                    nc.scalar.activation(
                        out=s[:, :W],
                        in_=t[:, :W],
                        func=mybir.ActivationFunctionType.Exp,
                        scale=_S,
                        bias=bias[:],
                        accum_out=dst,
                    )
                else:
                    assert W % 4 == 0
                    sl = nblk + blk * n_dve + idv[blk]
                    idv[blk] += 1
                    i16 = scr.tile([_P, wmax_d], mybir.dt.int16, tag="i16")
                    # int16(A*x + B) bit pattern ~= bf16 exp(S*x - S)
                    nc.vector.tensor_scalar(
                        out=i16[:, :W],
                        in0=t[:, :W],
                        scalar1=_SCH_A,
                        scalar2=_SCH_B,
                        op0=mybir.AluOpType.mult,
                        op1=mybir.AluOpType.add,
                    )
                    bf = i16[:, :W].bitcast(mybir.dt.bfloat16)
                    h = W // 2
                    q = W // 4
                    f1 = scr.tile([_P, wmax_d // 2], mybir.dt.bfloat16, tag="f1")
                    nc.vector.tensor_tensor(
                        out=f1[:, :h],
                        in0=bf[:, :h],
                        in1=bf[:, h:],
                        op=mybir.AluOpType.add,
                    )
                    f2 = scr.tile([_P, wmax_d // 4], mybir.dt.bfloat16, tag="f2")
                    nc.vector.tensor_tensor(
                        out=f2[:, :q],
                        in0=f1[:, :q],
                        in1=f1[:, q : 2 * q],
                        op=mybir.AluOpType.add,
                    )
                    nc.vector.reduce_sum(
                        out=res[:, sl : sl + 1],
                        in_=f2[:, :q],
                        axis=mybir.AxisListType.X,
                    )
                off += _P * W
            for blk in range(nblk):
                if n_act_by_blk[blk] > 1:
                    b0 = acc_base[blk]
                    nc.vector.reduce_sum(
                        out=res[:, blk : blk + 1],
                        in_=acc[:, b0 : b0 + n_act_by_blk[blk]],
                        axis=mybir.AxisListType.X,
                    )
            nc.sync.dma_start(out=out[:], in_=res[:])

    nc.compile()
    return nc


def _get_nc(B, Cs):
    key = (B, Cs)
    if key not in _nc_cache:
        _nc_cache[key] = _build_nc(B, Cs)
    return _nc_cache[key]


def _pack_shard(shard_f16, plan):
    """[B, Cs] fp16 -> flat blob layout matching the global plan DMA order."""
    B, Cs = shard_f16.shape
    nblk = B // _P
    cur = [0] * nblk
    parts = []
    for blk, W, _ in plan:
        rows = shard_f16[blk * _P : (blk + 1) * _P]
        parts.append(rows[:, cur[blk] : cur[blk] + W].ravel())
        cur[blk] += W
    return np.concatenate(parts)


def _device_row_sums(logits, trace=False):
    """Shard the class dim over 8 cores, run the bass kernel, return
    (row_sums[B] float64 = sum_c exp(S*logits - S), BassKernelResults)."""
    from concourse.bass_utils import run_bass_kernel_spmd

    B, C = logits.shape
    Bp = -(-B // _P) * _P  # pad rows to a multiple of 128
    Cp = -(-C // _NCORES) * _NCORES  # pad cols to a multiple of 8
    x16 = np.maximum(logits, _CLAMP).astype(np.float16)
    if Bp != B or Cp != C:
        padded = np.full((Bp, Cp), _CLAMP, dtype=np.float16)
        padded[:B, :C] = x16
        x16 = padded
    Cs = Cp // _NCORES
    nblk = Bp // _P
    plan = _global_plan(nblk, Cs)
    n_dve = sum(1 for _, _, e in plan if e == "D") // nblk
    nc = _get_nc(Bp, Cs)
    in_maps = [
        {"x": _pack_shard(x16[:, i * Cs : (i + 1) * Cs], plan)} for i in range(_NCORES)
    ]
    r = run_bass_kernel_spmd(nc, in_maps, core_ids=list(range(_NCORES)), trace=trace)
    total = np.zeros(Bp, np.float64)
    for res in r.results:
        arr = res["sums"].astype(np.float64)  # [128, nblk*(1+n_dve)]
        act = arr[:, :nblk].T.reshape(Bp)
        dve = arr[:, nblk:].reshape(_P, nblk, n_dve).sum(axis=2).T.reshape(Bp)
        total += act + _GAMMA * dve
    # The clamp floor contributes ~1.8e-35 per clamped element on the ACT
    # side and ~0 on the DVE side; both are below fp32 resolution of the
    # per-row sums (>= exp(0) for a max-logit near 1), so no correction.
    return total[:B], r


def kernel(logits, labels):
    logits = np.ascontiguousarray(np.asarray(logits, dtype=np.float32))
    labels_i = np.asarray(labels).astype(np.int64)
    B, C = logits.shape

    total, _ = _device_row_sums(logits)

    rows = np.arange(B)
    t = logits[rows, labels_i].astype(np.float64)
    # subtract what the device actually added for the label column (its fp16
    # value); the margin math itself uses the exact fp32 target.
    t16 = t.astype(np.float16).astype(np.float64)
    thresh = float(np.cos(np.pi - _M2))
    ang = np.arccos(np.clip(t, -1.0 + _EPS, 1.0 - _EPS))
    cos_m = np.cos(ang + _M2)
    theta = np.where(t > thresh, cos_m, -2.0 - cos_m)

    # replace the label column's exp term, all under the constant shift S
    corrected = total - np.exp(_S * t16 - _S) + np.exp(_S * theta - _S)
    loss_rows = _S + np.log(corrected) - _S * theta
    return np.array(loss_rows.mean(), dtype=np.float32)


# revision 20
# speedup vs baseline: 1.1026x; 1.0252x over previous
"""CombinedMarginLoss (ArcFace, m1=1, m2=0.5, m3=0, easy_margin) on 8 trn2 cores.

Math: loss = mean_b [ logsumexp_c(margin_logits[b,c]) - S*theta_b ] where
margin_logits[b,c] = S*logits[b,c] except the label column which is S*theta_b.

Because logits are cosine similarities in [-1, 1], S*x - S lies in [-128, 0],
so exp(S*x - S) never overflows in fp32 and the per-row sum-exp needs no max
pass: a single DMA-bound sweep per core suffices.  The class dimension is
sharded across the 8 cores (partial-FC style); each core returns its partial
per-row sum of exp(S*x - S).  The O(B) label gather, margin transform, and
log/mean epilogue are done on the host as part of unsharding.

Optimizations:
- Inputs are shipped to the device as fp16 (logits are in [-1,1]; the fp16
  rounding jitter of +-1.6% per exp term averages out over the ~1e3
  effective softmax terms per row, final loss error ~1e-5 relative).
- Host packs each core's shard into a flat buffer of [128, W] chunk blobs so
  every DMA reads one fully contiguous region at max HBM bandwidth.
- exp is computed 70% on ScalarE (hardware Exp with fused per-partition
  accum_out) and 30% on VectorE via the Schraudolph bit-trick
  (int32(A*x+B) reinterpreted as fp32 ~= exp(S*x-S)), whose +1.07% bias is
  removed by a calibrated host-side gamma. Both engines then hide entirely
  under the DMA stream.
- Values below the clamp (-0.25, i.e. exp < 2e-35) cannot affect the sum;
  the host clamps so the bit-trick's int never goes negative.
"""

import numpy as np

_S = 64.0
_M2 = 0.5
_EPS = 1e-7
_NCORES = 8
_P = 128  # SBUF partitions

_CLAMP = -0.25  # exp(64*-0.25 - 64) = 1.8e-35: far below fp32 sum resolution

_LOG2E = 1.4426950408889634
# bf16 variant of the bit trick: bf16 has fp32's 8-bit exponent, so
# int16(A*x + B) bitcast to bf16 ~= exp(S*x - S); int16 output lets the
# tensor_scalar run in the DVE 4x mode and bf16 tensor_tensor folds run 2x.
_SCH_A = _S * _LOG2E * 2.0**7
_SCH_C = 0.0434609
_SCH_B = 2.0**7 * (127.0 - _S * _LOG2E - _SCH_C)
# E[bit-trick exp / true exp] under exp-weighted uniform inputs; calibrated
# against float64 on-device (see calib.py); host divides it back out.
_GAMMA = 0.99029446  # HW-calibrated (CoreSim value differs: 0.99284518)

# per 128-row block: (width, engine) chunk list; class dim = 12500 per core.
# 52% ScalarE / 48% VectorE; small trailing ACT chunk trims the kernel tail.
_CHUNKS_12500 = [(3000, "D"), (2750, "A"), (3000, "D"), (2750, "A"), (1000, "A")]


def _global_plan(nblk, Cs):
    """DMA-ordered list of (blk, W, eng).  Big chunks stream first; two small
    ACT chunks land last so both engines drain as the stream ends."""
    if Cs == 12500 and nblk == 4:
        acfg = {0: [6500], 1: [6500], 2: [5500, 1000], 3: [5500, 1000]}
        order = []
        for blk in range(nblk):
            order.append((blk, acfg[blk][0], "A"))
            order.append((blk, 3000, "D"))
            order.append((blk, 3000, "D"))
        order.append((2, 1000, "A"))
        order.append((3, 1000, "A"))
        return order
    return [(blk, W, e) for blk in range(nblk) for (W, e) in _chunk_plan(Cs)]
# DVE implementation: "ttr" (tensor_tensor_reduce fold) | "fold" (tensor_tensor
# adds + reduce) | "i32red" (int32 bit-trick + fp32 reduce, the v4 path).
# NOTE: "ttr" with bf16 operands passes CoreSim but faults TRN2 hardware
# (NRT_EXEC_UNIT_UNRECOVERABLE) — do not use.
_DVE_IMPL = "fold"

_nc_cache = {}


def _chunk_plan(Cs):
    if Cs % 12500 == 0:
        return _CHUNKS_12500 * (Cs // 12500)
    # fallback: uniform ~6250-wide ACT-only chunks
    n = max(1, -(-Cs // 6250))
    while Cs % n:
        n += 1
    return [(Cs // n, "A")] * n


def _build_nc(B, Cs):
    """Bass/Tile program for one core: xflat[B*Cs] fp16 (blob layout) ->
    sums[128, nblk*(1+n_dve)]; col blk = ScalarE partial, col nblk+blk*n_dve+i
    = VectorE (bit-trick, pre-gamma) partials of sum_c exp(S*x[...] - S)."""
    import concourse.bacc as bacc
    import concourse.mybir as mybir
    from concourse.tile import TileContext

    nblk = B // _P
    plan = _global_plan(nblk, Cs)
    n_act_by_blk = [sum(1 for b, _, e in plan if b == k and e == "A") for k in range(nblk)]
    n_dve_by_blk = [sum(1 for b, _, e in plan if b == k and e == "D") for k in range(nblk)]
    n_dve = n_dve_by_blk[0]
    assert all(v == n_dve for v in n_dve_by_blk)
    acc_base = [sum(n_act_by_blk[:k]) for k in range(nblk)]
    n_acc = sum(n_act_by_blk)
    n_a_chunks = sum(1 for _, _, e in plan if e == "A")
    n_d_chunks = sum(1 for _, _, e in plan if e == "D")
    wmax = max([w for _, w, e in plan if e == "A"] or [1])
    wmax_d = max([w for _, w, e in plan if e == "D"] or [1])

    nc = bacc.Bacc("TRN2", target_bir_lowering=False)
    x = nc.dram_tensor("x", [B * Cs], mybir.dt.float16, kind="ExternalInput")
    out = nc.dram_tensor(
        "sums", [_P, nblk * (1 + n_dve)], mybir.dt.float32, kind="ExternalOutput"
    )

    with TileContext(nc) as tc:
        # one buffer per chunk in each engine's input pool: no DMA ever waits
        # on a tile release, so the FIFO Sync queue never head-of-line blocks.
        with (
            tc.tile_pool(name="inA", bufs=max(n_a_chunks, 1)) as inA,
            tc.tile_pool(name="inD", bufs=max(n_d_chunks, 1)) as inD,
            tc.tile_pool(name="scr", bufs=2) as scr,
            tc.tile_pool(name="acc", bufs=1) as accp,
        ):
            bias = accp.tile([_P, 1], mybir.dt.float32)
            nc.gpsimd.memset(bias[:], -_S)
            acc = accp.tile([_P, max(n_acc, 1)], mybir.dt.float32)
            res = accp.tile([_P, nblk * (1 + n_dve)], mybir.dt.float32)
            ia = [0] * nblk
            idv = [0] * nblk
            off = 0
            for blk, W, eng in plan:
                if eng == "A":
                    t = inA.tile([_P, wmax], mybir.dt.float16, tag="inA")
                else:
                    t = inD.tile([_P, wmax_d], mybir.dt.float16, tag="inD")
                nc.sync.dma_start(
                    out=t[:, :W],
                    in_=x[off : off + _P * W].rearrange("(p w) -> p w", p=_P),
                )
                if eng == "A":
                    s = scr.tile([_P, wmax], mybir.dt.float16, tag="scr")
                    if n_act_by_blk[blk] == 1:
                        dst = res[:, blk : blk + 1]
                    else:
                        j = acc_base[blk] + ia[blk]
                        dst = acc[:, j : j + 1]
                    ia[blk] += 1
                    # s = exp(S*t - S); dst = per-partition row-sum of s
                    nc.scalar.activation(
                        out=s[:, :W],
                        in_=t[:, :W],
                        func=mybir.ActivationFunctionType.Exp,
                        scale=_S,
                        bias=bias[:],
                        accum_out=dst,
                    )
                else:
                    assert W % 4 == 0
                    sl = nblk + blk * n_dve + idv[blk]
                    idv[blk] += 1
                    i16 = scr.tile([_P, wmax_d], mybir.dt.int16, tag="i16")
                    # int16(A*x + B) bit pattern ~= bf16 exp(S*x - S)
                    nc.vector.tensor_scalar(
                        out=i16[:, :W],
                        in0=t[:, :W],
                        scalar1=_SCH_A,
                        scalar2=_SCH_B,
                        op0=mybir.AluOpType.mult,
                        op1=mybir.AluOpType.add,
                    )
                    bf = i16[:, :W].bitcast(mybir.dt.bfloat16)
                    h = W // 2
                    q = W // 4
                    f1 = scr.tile([_P, wmax_d // 2], mybir.dt.bfloat16, tag="f1")
                    nc.vector.tensor_tensor(
                        out=f1[:, :h],
                        in0=bf[:, :h],
                        in1=bf[:, h:],
                        op=mybir.AluOpType.add,
                    )
                    f2 = scr.tile([_P, wmax_d // 4], mybir.dt.bfloat16, tag="f2")
                    nc.vector.tensor_tensor(
                        out=f2[:, :q],
                        in0=f1[:, :q],
                        in1=f1[:, q : 2 * q],
                        op=mybir.AluOpType.add,
                    )
                    nc.vector.reduce_sum(
                        out=res[:, sl : sl + 1],
                        in_=f2[:, :q],
                        axis=mybir.AxisListType.X,
                    )
                off += _P * W
            for blk in range(nblk):
                if n_act_by_blk[blk] > 1:
                    b0 = acc_base[blk]
                    nc.vector.reduce_sum(
                        out=res[:, blk : blk + 1],
                        in_=acc[:, b0 : b0 + n_act_by_blk[blk]],
                        axis=mybir.AxisListType.X,
                    )
            nc.sync.dma_start(out=out[:], in_=res[:])

    nc.compile()
    return nc


def _get_nc(B, Cs):
    key = (B, Cs)
    if key not in _nc_cache:
        _nc_cache[key] = _build_nc(B, Cs)
    return _nc_cache[key]


def _pack_shard(shard_f16, plan):
    """[B, Cs] fp16 -> flat blob layout matching the global plan DMA order."""
    B, Cs = shard_f16.shape
    nblk = B // _P
    cur = [0] * nblk
    parts = []
    for blk, W, _ in plan:
        rows = shard_f16[blk * _P : (blk + 1) * _P]
        parts.append(rows[:, cur[blk] : cur[blk] + W].ravel())
        cur[blk] += W
    return np.concatenate(parts)


def _device_row_sums(logits, trace=False):
    """Shard the class dim over 8 cores, run the bass kernel, return
    (row_sums[B] float64 = sum_c exp(S*logits - S), BassKernelResults)."""
    from concourse.bass_utils import run_bass_kernel_spmd

    B, C = logits.shape
    Bp = -(-B // _P) * _P  # pad rows to a multiple of 128
    Cp = -(-C // _NCORES) * _NCORES  # pad cols to a multiple of 8
    x16 = np.maximum(logits, _CLAMP).astype(np.float16)
    if Bp != B or Cp != C:
        padded = np.full((Bp, Cp), _CLAMP, dtype=np.float16)
        padded[:B, :C] = x16
        x16 = padded
    Cs = Cp // _NCORES
    nblk = Bp // _P
    plan = _global_plan(nblk, Cs)
    n_dve = sum(1 for _, _, e in plan if e == "D") // nblk
    nc = _get_nc(Bp, Cs)
    in_maps = [
        {"x": _pack_shard(x16[:, i * Cs : (i + 1) * Cs], plan)} for i in range(_NCORES)
    ]
    r = run_bass_kernel_spmd(nc, in_maps, core_ids=list(range(_NCORES)), trace=trace)
    total = np.zeros(Bp, np.float64)
    for res in r.results:
        arr = res["sums"].astype(np.float64)  # [128, nblk*(1+n_dve)]
        act = arr[:, :nblk].T.reshape(Bp)
        dve = arr[:, nblk:].reshape(_P, nblk, n_dve).sum(axis=2).T.reshape(Bp)
        total += act + _GAMMA * dve
    # The clamp floor contributes ~1.8e-35 per clamped element on the ACT
    # side and ~0 on the DVE side; both are below fp32 resolution of the
    # per-row sums (>= exp(0) for a max-logit near 1), so no correction.
    return total[:B], r


def kernel(logits, labels):
    logits = np.ascontiguousarray(np.asarray(logits, dtype=np.float32))
    labels_i = np.asarray(labels).astype(np.int64)
    B, C = logits.shape

    total, _ = _device_row_sums(logits)

    rows = np.arange(B)
    t = logits[rows, labels_i].astype(np.float64)
    # subtract what the device actually added for the label column (its fp16
    # value); the margin math itself uses the exact fp32 target.
    t16 = t.astype(np.float16).astype(np.float64)
    thresh = float(np.cos(np.pi - _M2))
    ang = np.arccos(np.clip(t, -1.0 + _EPS, 1.0 - _EPS))
    cos_m = np.cos(ang + _M2)
    theta = np.where(t > thresh, cos_m, -2.0 - cos_m)

    # replace the label column's exp term, all under the constant shift S
    corrected = total - np.exp(_S * t16 - _S) + np.exp(_S * theta - _S)
    loss_rows = _S + np.log(corrected) - _S * theta
    return np.array(loss_rows.mean(), dtype=np.float32)


# revision 21
# speedup vs baseline: 1.2279x; 1.1136x over previous
"""CombinedMarginLoss (ArcFace, m1=1, m2=0.5, m3=0, easy_margin) on 8 trn2 cores.

Math: loss = mean_b [ logsumexp_c(margin_logits[b,c]) - S*theta_b ] where
margin_logits[b,c] = S*logits[b,c] except the label column which is S*theta_b.

Because logits are cosine similarities in [-1, 1], S*x - S lies in [-128, 0],
so exp(S*x - S) never overflows in fp32 and the per-row sum-exp needs no max
pass: a single DMA-bound sweep per core suffices.  The class dimension is
sharded across the 8 cores (partial-FC style); each core returns its partial
per-row sum of exp(S*x - S).  The O(B) label gather, margin transform, and
log/mean epilogue are done on the host as part of unsharding.

Optimizations:
- Inputs are shipped to the device as fp16 (logits are in [-1,1]; the fp16
  rounding jitter of +-1.6% per exp term averages out over the ~1e3
  effective softmax terms per row, final loss error ~1e-5 relative).
- Host packs each core's shard into a flat buffer of [128, W] chunk blobs so
  every DMA reads one fully contiguous region at max HBM bandwidth.
- exp is computed 70% on ScalarE (hardware Exp with fused per-partition
  accum_out) and 30% on VectorE via the Schraudolph bit-trick
  (int32(A*x+B) reinterpreted as fp32 ~= exp(S*x-S)), whose +1.07% bias is
  removed by a calibrated host-side gamma. Both engines then hide entirely
  under the DMA stream.
- Values below the clamp (-0.25, i.e. exp < 2e-35) cannot affect the sum;
  the host clamps so the bit-trick's int never goes negative.
"""

import numpy as np

_S = 64.0
_M2 = 0.5
_EPS = 1e-7
_NCORES = 8
_P = 128  # SBUF partitions

_CLAMP = -0.25  # exp(64*-0.25 - 64) = 1.8e-35: far below fp32 sum resolution

_LOG2E = 1.4426950408889634
# bf16 variant of the bit trick: bf16 has fp32's 8-bit exponent, so
# int16(A*x + B) bitcast to bf16 ~= exp(S*x - S); int16 output lets the
# tensor_scalar run in the DVE 4x mode and bf16 tensor_tensor folds run 2x.
_SCH_A = _S * _LOG2E * 2.0**7
_SCH_C = 0.0434609
_SCH_B = 2.0**7 * (127.0 - _S * _LOG2E - _SCH_C)
# E[bit-trick exp / true exp] under exp-weighted uniform inputs; calibrated
# against float64 on-device (see calib.py); host divides it back out.
_GAMMA = 0.99029446  # HW-calibrated (CoreSim value differs: 0.99284518)

# per 128-row block: (width, engine) chunk list; class dim = 12500 per core.
# 52% ScalarE / 48% VectorE; small trailing ACT chunk trims the kernel tail.
_CHUNKS_12500 = [(3000, "D"), (2750, "A"), (3000, "D"), (2750, "A"), (1000, "A")]


def _global_plan(nblk, Cs):
    """DMA-ordered list of (blk, W, eng).  Rounds of (A3750, D2500) keep both
    engines continuously fed; the last block tapers into small chunks so both
    engines drain as the stream ends."""
    if Cs == 12500 and nblk == 4:
        order = []
        for blk in range(3):
            order += [
                (blk, 3750, "A"),
                (blk, 2500, "D"),
                (blk, 3750, "A"),
                (blk, 2500, "D"),
            ]
        order += [
            (3, 3750, "A"),
            (3, 2500, "D"),
            (3, 2250, "A"),
            (3, 1500, "D"),
            (3, 1500, "A"),
            (3, 1000, "D"),
        ]
        return order
    return [(blk, W, e) for blk in range(nblk) for (W, e) in _chunk_plan(Cs)]


# DVE implementation: "ttr" (tensor_tensor_reduce fold) | "fold" (tensor_tensor
# adds + reduce) | "i32red" (int32 bit-trick + fp32 reduce, the v4 path).
# NOTE: "ttr" with bf16 operands passes CoreSim but faults TRN2 hardware
# (NRT_EXEC_UNIT_UNRECOVERABLE) — do not use.
_DVE_IMPL = "fold"

_nc_cache = {}


def _chunk_plan(Cs):
    if Cs % 12500 == 0:
        return _CHUNKS_12500 * (Cs // 12500)
    # fallback: uniform ~6250-wide ACT-only chunks
    n = max(1, -(-Cs // 6250))
    while Cs % n:
        n += 1
    return [(Cs // n, "A")] * n


def _build_nc(B, Cs):
    """Bass/Tile program for one core: xflat[B*Cs] fp16 (blob layout) ->
    sums[128, nblk*(1+n_dve)]; col blk = ScalarE partial, col nblk+blk*n_dve+i
    = VectorE (bit-trick, pre-gamma) partials of sum_c exp(S*x[...] - S)."""
    import concourse.bacc as bacc
    import concourse.mybir as mybir
    from concourse.tile import TileContext

    nblk = B // _P
    plan = _global_plan(nblk, Cs)
    n_act_by_blk = [sum(1 for b, _, e in plan if b == k and e == "A") for k in range(nblk)]
    n_dve_by_blk = [sum(1 for b, _, e in plan if b == k and e == "D") for k in range(nblk)]
    d_base = [sum(n_dve_by_blk[:k]) for k in range(nblk)]
    n_d_slots = sum(n_dve_by_blk)
    acc_base = [sum(n_act_by_blk[:k]) for k in range(nblk)]
    n_acc = sum(n_act_by_blk)
    n_a_chunks = sum(1 for _, _, e in plan if e == "A")
    n_d_chunks = sum(1 for _, _, e in plan if e == "D")
    wmax = max([w for _, w, e in plan if e == "A"] or [1])
    wmax_d = max([w for _, w, e in plan if e == "D"] or [1])

    nc = bacc.Bacc("TRN2", target_bir_lowering=False)
    x = nc.dram_tensor("x", [B * Cs], mybir.dt.float16, kind="ExternalInput")
    out = nc.dram_tensor(
        "sums", [_P, nblk + n_d_slots], mybir.dt.float32, kind="ExternalOutput"
    )

    with TileContext(nc) as tc:
        # one buffer per chunk in each engine's input pool: no DMA ever waits
        # on a tile release, so the FIFO Sync queue never head-of-line blocks.
        with (
            tc.tile_pool(name="inA", bufs=max(n_a_chunks, 1)) as inA,
            tc.tile_pool(name="inD", bufs=max(n_d_chunks, 1)) as inD,
            tc.tile_pool(name="scr", bufs=2) as scr,
            tc.tile_pool(name="acc", bufs=1) as accp,
        ):
            bias = accp.tile([_P, 1], mybir.dt.float32)
            nc.gpsimd.memset(bias[:], -_S)
            acc = accp.tile([_P, max(n_acc, 1)], mybir.dt.float32)
            res = accp.tile([_P, nblk + n_d_slots], mybir.dt.float32)
            ia = [0] * nblk
            idv = [0] * nblk
            off = 0
            for blk, W, eng in plan:
                if eng == "A":
                    t = inA.tile([_P, wmax], mybir.dt.float16, tag="inA")
                else:
                    t = inD.tile([_P, wmax_d], mybir.dt.float16, tag="inD")
                nc.sync.dma_start(
                    out=t[:, :W],
                    in_=x[off : off + _P * W].rearrange("(p w) -> p w", p=_P),
                )
                if eng == "A":
                    s = scr.tile([_P, wmax], mybir.dt.float16, tag="scr")
                    if n_act_by_blk[blk] == 1:
                        dst = res[:, blk : blk + 1]
                    else:
                        j = acc_base[blk] + ia[blk]
                        dst = acc[:, j : j + 1]
                    ia[blk] += 1
                    # s = exp(S*t - S); dst = per-partition row-sum of s
                    nc.scalar.activation(
                        out=s[:, :W],
                        in_=t[:, :W],
                        func=mybir.ActivationFunctionType.Exp,
                        scale=_S,
                        bias=bias[:],
                        accum_out=dst,
                    )
                else:
                    assert W % 4 == 0
                    sl = nblk + d_base[blk] + idv[blk]
                    idv[blk] += 1
                    i16 = scr.tile([_P, wmax_d], mybir.dt.int16, tag="i16")
                    # int16(A*x + B) bit pattern ~= bf16 exp(S*x - S)
                    nc.vector.tensor_scalar(
                        out=i16[:, :W],
                        in0=t[:, :W],
                        scalar1=_SCH_A,
                        scalar2=_SCH_B,
                        op0=mybir.AluOpType.mult,
                        op1=mybir.AluOpType.add,
                    )
                    bf = i16[:, :W].bitcast(mybir.dt.bfloat16)
                    h = W // 2
                    q = W // 4
                    f1 = scr.tile([_P, wmax_d // 2], mybir.dt.bfloat16, tag="f1")
                    nc.vector.tensor_tensor(
                        out=f1[:, :h],
                        in0=bf[:, :h],
                        in1=bf[:, h:],
                        op=mybir.AluOpType.add,
                    )
                    f2 = scr.tile([_P, wmax_d // 4], mybir.dt.bfloat16, tag="f2")
                    nc.vector.tensor_tensor(
                        out=f2[:, :q],
                        in0=f1[:, :q],
                        in1=f1[:, q : 2 * q],
                        op=mybir.AluOpType.add,
                    )
                    nc.vector.reduce_sum(
                        out=res[:, sl : sl + 1],
                        in_=f2[:, :q],
                        axis=mybir.AxisListType.X,
                    )
                off += _P * W
            for blk in range(nblk):
                if n_act_by_blk[blk] > 1:
                    b0 = acc_base[blk]
                    nc.vector.reduce_sum(
                        out=res[:, blk : blk + 1],
                        in_=acc[:, b0 : b0 + n_act_by_blk[blk]],
                        axis=mybir.AxisListType.X,
                    )
            nc.sync.dma_start(out=out[:], in_=res[:])

    nc.compile()
    return nc


def _get_nc(B, Cs):
    key = (B, Cs)
    if key not in _nc_cache:
        _nc_cache[key] = _build_nc(B, Cs)
    return _nc_cache[key]


def _pack_shard(shard_f16, plan):
    """[B, Cs] fp16 -> flat blob layout matching the global plan DMA order."""
    B, Cs = shard_f16.shape
    nblk = B // _P
    cur = [0] * nblk
    parts = []
    for blk, W, _ in plan:
        rows = shard_f16[blk * _P : (blk + 1) * _P]
        parts.append(rows[:, cur[blk] : cur[blk] + W].ravel())
        cur[blk] += W
    return np.concatenate(parts)


def _device_row_sums(logits, trace=False):
    """Shard the class dim over 8 cores, run the bass kernel, return
    (row_sums[B] float64 = sum_c exp(S*logits - S), BassKernelResults)."""
    from concourse.bass_utils import run_bass_kernel_spmd

    B, C = logits.shape
    Bp = -(-B // _P) * _P  # pad rows to a multiple of 128
    Cp = -(-C // _NCORES) * _NCORES  # pad cols to a multiple of 8
    x16 = np.maximum(logits, _CLAMP).astype(np.float16)
    if Bp != B or Cp != C:
        padded = np.full((Bp, Cp), _CLAMP, dtype=np.float16)
        padded[:B, :C] = x16
        x16 = padded
    Cs = Cp // _NCORES
    nblk = Bp // _P
    plan = _global_plan(nblk, Cs)
    n_dve_by_blk = [sum(1 for b, _, e in plan if b == k and e == "D") for k in range(nblk)]
    d_base = [sum(n_dve_by_blk[:k]) for k in range(nblk)]
    nc = _get_nc(Bp, Cs)
    in_maps = [
        {"x": _pack_shard(x16[:, i * Cs : (i + 1) * Cs], plan)} for i in range(_NCORES)
    ]
    r = run_bass_kernel_spmd(nc, in_maps, core_ids=list(range(_NCORES)), trace=trace)
    total = np.zeros(Bp, np.float64)
    for res in r.results:
        arr = res["sums"].astype(np.float64)  # [128, nblk + n_d_slots]
        act = arr[:, :nblk].T.reshape(Bp)
        dve = np.zeros_like(act)
        for blk in range(nblk):
            lo = nblk + d_base[blk]
            dve[blk * _P : (blk + 1) * _P] = arr[:, lo : lo + n_dve_by_blk[blk]].sum(
                axis=1
            )
        total += act + _GAMMA * dve
    # The clamp floor contributes ~1.8e-35 per clamped element on the ACT
    # side and ~0 on the DVE side; both are below fp32 resolution of the
    # per-row sums (>= exp(0) for a max-logit near 1), so no correction.
    return total[:B], r


def kernel(logits, labels):
    logits = np.ascontiguousarray(np.asarray(logits, dtype=np.float32))
    labels_i = np.asarray(labels).astype(np.int64)
    B, C = logits.shape

    total, _ = _device_row_sums(logits)

    rows = np.arange(B)
    t = logits[rows, labels_i].astype(np.float64)
    # subtract what the device actually added for the label column (its fp16
    # value); the margin math itself uses the exact fp32 target.
    t16 = t.astype(np.float16).astype(np.float64)
    thresh = float(np.cos(np.pi - _M2))
    ang = np.arccos(np.clip(t, -1.0 + _EPS, 1.0 - _EPS))
    cos_m = np.cos(ang + _M2)
    theta = np.where(t > thresh, cos_m, -2.0 - cos_m)

    # replace the label column's exp term, all under the constant shift S
    corrected = total - np.exp(_S * t16 - _S) + np.exp(_S * theta - _S)
    loss_rows = _S + np.log(corrected) - _S * theta
    return np.array(loss_rows.mean(), dtype=np.float32)
